# revision 1
# baseline (speedup 1.0000x reference)
"""Trainium2 Bass kernel: normalized min-sum LDPC decoder (nn_Decoding_model).

Sharding: pure batch data-parallelism. B=16 rows split across 8 NeuronCores
(2 rows/core); H-derived matrices are replicated per core.

Per core, per iteration (BL=2 batch rows):
  A_bcast[b] : [128, N] f32, each partition holds |so_b| (PE transpose of the
               column-major state + DRAM-broadcast DMA)
  neg_v      : per m-chunk [128, N]: hbig(fp16, NEG_BACK on non-edges) - A_bcast
  vmax8      : DVE top-8 of neg_v per check row -> m1=-v0, m2=-v1 (dup-exact)
  onehot     : (neg_v == v0) bf16  (argmin edges; ties give t2=0, harmless)
  parity     : P[m] = sum_n H^T[n,m]*(so[n]<0)  (fp8 matmul, exact counts)
  rs = 1-2*(P mod 2);  t1 = rs*m1;  t2 = rs*(m2-m1)  (3-way bf16 splits)
  main[n]    : sum_m H[m,n]*t1[m]       (bf16 H stationary, fp32 PSUM)
  corr[n]    : sum_m onehot[m,n]*t2[m]  (bf16 onehot stationary)
  so_new     : si + softplus(w)*sign(so)*(main+corr)

Layout "cm" = column-major [128, NC]: n = c*128 + p.
"""

from contextlib import ExitStack

import numpy as np

import concourse.bass as bass
import concourse.mybir as mybir
import concourse.tile as tile
from concourse import bacc, bass_utils
F32 = mybir.dt.float32
F16 = mybir.dt.float16
BF16 = mybir.dt.bfloat16
F8 = mybir.dt.float8e4
F8E5 = mybir.dt.float8e5
I32 = mybir.dt.int32
OP = mybir.AluOpType

NEG_BACK = -57344.0  # "minus infinity" for non-edges (fp8e5-exact: 1.75*2^15, dominates |so|)

B, M, N, IT = 16, 1024, 2048, 5
N_CORES = 8
BL = B // N_CORES


def build(nc: bass.Bass, M=M, N=N, BL=BL, IT=IT, oh_bufs=10, gp_tt=7, oh_act=8, oh_gp=0, nv_bufs=4, ohs_bufs=2, st_bufs=2, p2_dve=0, halves=1, skip=()):
    MC = M // 128  # m-chunks
    NC = N // 128  # n-chunks

    d_si = nc.dram_tensor("si_cm", [128, NC * BL], F32, kind="ExternalInput").ap()
    d_h = nc.dram_tensor("h_bf", [M, N], BF16, kind="ExternalInput").ap()
    d_hb = nc.dram_tensor("h_big", [M, N], F8E5, kind="ExternalInput").ap()
    d_ht = nc.dram_tensor("h_t", [N, M], F8, kind="ExternalInput").ap()
    d_misc = nc.dram_tensor("misc_in", [128, 129], F32, kind="ExternalInput").ap()
    d_abc0 = nc.dram_tensor("abc0", [128, N * BL], F32, kind="ExternalInput").ap()
    d_out = nc.dram_tensor("out", [BL, N], F32, kind="ExternalOutput").ap()

    with tile.TileContext(nc) as tc, ExitStack() as ctx:
        const = ctx.enter_context(tc.tile_pool(name="const", bufs=1))
        state_p = ctx.enter_context(tc.tile_pool(name="state", bufs=st_bufs))
        negv_p = ctx.enter_context(tc.tile_pool(name="negv", bufs=nv_bufs))
        oh_p = ctx.enter_context(tc.tile_pool(name="oh", bufs=oh_bufs))
        ohs_p = ctx.enter_context(tc.tile_pool(name="ohs", bufs=ohs_bufs))
        psum_p = ctx.enter_context(tc.tile_pool(name="ps", bufs=1, space="PSUM"))
        pstr_p = ctx.enter_context(tc.tile_pool(name="pstr", bufs=2, space="PSUM"))
        dram_p = ctx.enter_context(tc.tile_pool(name="dram", bufs=1, space="DRAM"))

        # ---- persistent loads ----
        t_h = const.tile([128, MC * N], BF16)  # H, m-chunk mc at cols [mc*N,(mc+1)*N)
        t_hb = const.tile([128, MC * N], F8E5)  # (1-H)*NEG_BACK
        t_ht = const.tile([128, NC * M], F8)  # H^T, n-chunk c at cols [c*M,(c+1)*M)
        misc = const.tile([128, 129], F32)  # ident(128) | norm(1)
        ident = misc[:, 0:128]
        t_norm = misc[:, 128:129]
        nc.sync.dma_start(misc[:], d_misc)
        # dummy matmul so the PE observes misc's DMA sem before any transpose
        # (transpose-mode matmuls only support a single sync wait)
        pdum = pstr_p.tile([1, 1], F32, tag="dum", name="dum", bufs=1)
        nc.tensor.matmul(pdum[:], lhsT=ident[:, 0:1], rhs=ident[:, 0:1], start=True, stop=True)
        hb_r = d_hb.rearrange("(mc p) n -> mc p n", p=128)
        h_r = d_h.rearrange("(mc p) n -> mc p n", p=128)
        ht_r = d_ht.rearrange("(c p) m -> c p m", p=128)
        for mc in range(MC):
            nc.sync.dma_start(t_hb[:, mc * N : (mc + 1) * N], hb_r[mc])
        for c in range(NC):
            nc.sync.dma_start(t_ht[:, c * M : (c + 1) * M], ht_r[c])
        for mc in range(MC):
            nc.sync.dma_start(t_h[:, mc * N : (mc + 1) * N], h_r[mc])

        t_si = const.tile([128, NC * BL], F32)
        nc.sync.dma_start(t_si[:], d_si)
        si = [t_si[:, NC * b : NC * (b + 1)] for b in range(BL)]

        abc_p = ctx.enter_context(tc.tile_pool(name="abcp", bufs=2))
        abc = [None] * BL
        at_sb = [const.tile([NC, 128], F32, tag=f"atsb{b}", name=f"atsb{b}") for b in range(BL)]
        d_arow = dram_p.tile([BL, N], F32, name="d_arow")

        def derive_state(so_ap, b, negs):
            """From so (cm [128, NC]) write A, S (f32) and neg (fp8, col 2c+b)."""
            st = state_p.tile([128, 2 * NC], F32, tag=f"st{b}", name=f"st{b}")
            A = st[:, 0:NC]
            S = st[:, NC : 2 * NC]
            nc.vector.tensor_scalar(
                out=A.bitcast(I32), in0=so_ap.bitcast(I32),
                scalar1=0x7FFFFFFF, scalar2=None, op0=OP.bitwise_and,
            )
            nc.vector.tensor_scalar(out=S, in0=so_ap, scalar1=0.0, scalar2=2.0, op0=OP.is_ge, op1=OP.mult)
            nc.vector.tensor_scalar(out=S, in0=S, scalar1=-1.0, scalar2=None, op0=OP.add)
            nc.vector.tensor_scalar(
                out=negs[:].rearrange("p (c two) -> p c two", two=2)[:, :, b : b + 1],
                in0=so_ap.unsqueeze(2),
                scalar1=0.0, scalar2=None, op0=OP.is_lt,
            )
            return A, S

        def bcast_A(A, b):
            """A (cm [128, NC]) -> abc[b] [128, N] row-major broadcast."""
            pt = pstr_p.tile([NC, 128], F32, tag="tr", name="tr")
            nc.tensor.transpose(pt[:], A, ident)
            nc.vector.tensor_copy(at_sb[b][:], pt[:])
            nc.sync.dma_start(d_arow[b : b + 1, :], at_sb[b][:])
            abc[b] = abc_p.tile([128, N], F32, tag=f"abc{b}", name=f"abc{b}")
            # split across DMA queues: one 1MB broadcast would serialize on a
            # single queue's bandwidth
            for q in range(4):
                nc.sync.dma_start(
                    abc[b][:, q * (N // 4) : (q + 1) * (N // 4)],
                    d_arow[b : b + 1, q * (N // 4) : (q + 1) * (N // 4)].to_broadcast([128, N // 4]),
                )

        # ---- init state from si ----
        so = [si[b] for b in range(BL)]
        negs = state_p.tile([128, 2 * NC], F8, tag="negs", name="negs")
        AS = [derive_state(so[b], b, negs) for b in range(BL)]
        for b in range(BL):
            # iteration-0 A broadcast comes precomputed from the host
            abc[b] = abc_p.tile([128, N], F32, tag=f"abc{b}", name=f"abc{b}")
            for q in range(2):
                nc.sync.dma_start(
                    abc[b][:, q * (N // 2) : (q + 1) * (N // 2)],
                    d_abc0[:, N * b + q * (N // 2) : N * b + (q + 1) * (N // 2)],
                )

        for it in range(IT):
            # parity: P[m-part, 2*mc+b] = sum_n H^T * neg   (PE, early)
            pp = psum_p.tile([128, 2 * MC], F32, tag="pp", name="pp")
            for mc in range(MC if "parity" not in skip else 0):
                for c in range(NC):
                    nc.tensor.matmul(
                        pp[:, 2 * mc : 2 * mc + 2],
                        lhsT=t_ht[:, c * M + 128 * mc : c * M + 128 * (mc + 1)],
                        rhs=negs[:, 2 * c : 2 * c + 2],
                        start=(c == 0),
                        stop=(c == NC - 1),
                    )
            # smalls: rs | t1 | t2 | tmp | tmp2 (f32); t1/t2 h,m,l (bf16)
            sm = state_p.tile([128, 10 * MC], F32, tag="sm", name="sm")
            rs, t1s, t2s = sm[:, : 2 * MC], sm[:, 2 * MC : 4 * MC], sm[:, 4 * MC : 6 * MC]
            tmp = sm[:, 6 * MC : 8 * MC]
            tmp2 = sm[:, 8 * MC : 10 * MC]
            smb = state_p.tile([128, 12 * MC], BF16, tag="smb", name="smb")
            t1g = [smb[:, 2 * g * MC : 2 * (g + 1) * MC] for g in range(3)]
            t2g = [smb[:, 2 * (g + 3) * MC : 2 * (g + 4) * MC] for g in range(3)]
            ri = state_p.tile([128, 2 * MC], I32, tag="ri", name="ri")
            nc.vector.tensor_copy(ri[:], pp[:])  # exact: P is integer-valued
            nc.vector.tensor_scalar(out=ri[:], in0=ri[:], scalar1=1, scalar2=None, op0=OP.bitwise_and)
            nc.vector.tensor_copy(rs, ri[:])
            nc.vector.tensor_scalar(out=rs, in0=rs, scalar1=-2.0, scalar2=1.0, op0=OP.mult, op1=OP.add)

            vmax = [state_p.tile([128, 8 * MC], F32, tag=f"vm{b}", name=f"vm{b}") for b in range(BL)]
            smb_g = smb[:].rearrange("p (g j) -> p j g", g=6)
            new_negs = state_p.tile([128, 2 * NC], F8, tag="negs", name="negs") if it < IT - 1 else None
            new_so, new_AS = [], []
            for b in range(BL):
                ohs = {}
                for mc in range(MC):
                    nv = negv_p.tile([128, N], F32, tag="nv", name="nv")
                    tt_eng = nc.gpsimd if mc < gp_tt else nc.vector
                    tt_eng.tensor_tensor(
                        out=nv[:], in0=t_hb[:, mc * N : (mc + 1) * N], in1=abc[b][:], op=OP.subtract
                    )
                    if "max8" not in skip:
                        nc.vector.max(out=vmax[b][:, 8 * mc : 8 * mc + 8], in_=nv[:])
                    oh = oh_p.tile([128, N], BF16, tag="oh", name="oh")
                    ohs[mc] = oh
                    if "oh" not in skip:
                        # onehot on the scalar engine:
                        # s = Sign(vmax0 - nv) in {0 (argmin), 1}; oh = 1 - s
                        osg = ohs_p.tile([128, N], BF16, tag="osg", name="osg")
                        nc.scalar.activation(
                            osg[:], nv[:], mybir.ActivationFunctionType.Sign,
                            bias=vmax[b][:, 8 * mc : 8 * mc + 1], scale=-1.0,
                        )
                        if mc < p2_dve:
                            nc.vector.tensor_scalar(
                                out=oh[:], in0=osg[:], scalar1=-1.0, scalar2=1.0,
                                op0=OP.mult, op1=OP.add,
                            )
                        else:
                            nc.scalar.activation(
                                oh[:], osg[:], mybir.ActivationFunctionType.Copy,
                                bias=1.0, scale=-1.0,
                            )
                # t1/t2 + 3-way bf16 splits, in mc-halves so the PE
                # accumulation matmuls can start before the last max8
                vm8 = vmax[b][:].rearrange("p (c k) -> p c k", k=8)
                rs_bv = rs[:].rearrange("p (c two) -> p c two", two=2)
                for h in range(halves):
                    lo, hi = (MC * h) // halves, (MC * (h + 1)) // halves
                    t2s_bv = t2s.rearrange("p (c two) -> p c two", two=2)[:, lo:hi, b : b + 1]
                    nc.vector.tensor_tensor(
                        out=t2s_bv, in0=vm8[:, lo:hi, 0:1], in1=vm8[:, lo:hi, 1:2], op=OP.subtract
                    )
                    nc.vector.tensor_tensor(
                        out=t2s_bv, in0=t2s_bv, in1=rs_bv[:, lo:hi, b : b + 1], op=OP.mult
                    )
                    nc.vector.scalar_tensor_tensor(
                        out=t1s[:].rearrange("p (c two) -> p c two", two=2)[:, lo:hi, b : b + 1],
                        in0=vm8[:, lo:hi, 0:1],
                        scalar=-1.0,
                        in1=rs_bv[:, lo:hi, b : b + 1],
                        op0=OP.mult, op1=OP.mult,
                    )
                    for ts_, tmp_, gs in ((t1s, tmp, t1g), (t2s, tmp2, t2g)):
                        ts_b = ts_.rearrange("p (c two) -> p c two", two=2)[:, lo:hi, b : b + 1]
                        tm_b = tmp_.rearrange("p (c two) -> p c two", two=2)[:, lo:hi, b : b + 1]
                        g_b = [g.rearrange("p (c two) -> p c two", two=2)[:, lo:hi, b : b + 1] for g in gs]
                        nc.vector.tensor_copy(g_b[0], ts_b)
                        nc.vector.tensor_tensor(out=tm_b, in0=ts_b, in1=g_b[0], op=OP.subtract)
                        nc.vector.tensor_copy(g_b[1], tm_b)
                        nc.vector.tensor_tensor(out=g_b[2], in0=tm_b, in1=g_b[1], op=OP.subtract)

                # main(b): pm_b[n-part, 3c+k] = sum_m H*t1{h,m,l}[b]
                pm = psum_p.tile([128, 3 * NC], F32, tag=f"pm{b}", name=f"pm{b}")
                for c in range(NC if "main" not in skip else 0):
                    for mc in range(MC):
                        nc.tensor.matmul(
                            pm[:, 3 * c : 3 * c + 3],
                            lhsT=t_h[:, mc * N + 128 * c : mc * N + 128 * (c + 1)],
                            rhs=smb_g[:, 2 * mc + b : 2 * mc + b + 1, 0:3],
                            start=(mc == 0),
                            stop=(mc == MC - 1),
                        )
                # corr(b): pcr_b[n-part, 3c+k] = sum_m onehot*t2{h,m,l}
                pcr = psum_p.tile([128, 3 * NC], F32, tag=f"pcr{b}", name=f"pcr{b}")
                for c in range(NC if "corr" not in skip else 0):
                    for mc in range(MC):
                        nc.tensor.matmul(
                            pcr[:, 3 * c : 3 * c + 3],
                            lhsT=ohs[mc][:, 128 * c : 128 * (c + 1)],
                            rhs=smb_g[:, 2 * mc + b : 2 * mc + b + 1, 3:6],
                            start=(mc == 0),
                            stop=(mc == MC - 1),
                        )

                # combine(b)
                A_old, S_old = AS[b]
                st2 = state_p.tile([128, 3 * NC], F32, tag=f"cmb{b}", name=f"cmb{b}")
                crr, c1, so_n = st2[:, :NC], st2[:, NC : 2 * NC], st2[:, 2 * NC :]
                pcr_v = pcr[:].rearrange("p (c three) -> p c three", three=3)
                pm_v = pm[:].rearrange("p (c three) -> p c three", three=3)
                # accumulate one PSUM operand at a time (ISA: <=1 PSUM input/op)
                nc.vector.tensor_copy(crr.unsqueeze(2), pcr_v[:, :, 0:1])
                for k in (1, 2):
                    nc.vector.tensor_tensor(
                        out=crr.unsqueeze(2), in0=crr.unsqueeze(2),
                        in1=pcr_v[:, :, k : k + 1], op=OP.add,
                    )
                nc.vector.tensor_tensor(
                    out=c1.unsqueeze(2), in0=crr.unsqueeze(2), in1=pm_v[:, :, 0:1], op=OP.add
                )
                for k in (1, 2):
                    nc.vector.tensor_tensor(
                        out=c1.unsqueeze(2), in0=c1.unsqueeze(2), in1=pm_v[:, :, k : k + 1], op=OP.add
                    )
                nc.vector.scalar_tensor_tensor(
                    out=c1, in0=c1, scalar=t_norm, in1=S_old, op0=OP.mult, op1=OP.mult
                )
                nc.vector.tensor_tensor(out=so_n, in0=si[b], in1=c1, op=OP.add)
                new_so.append(so_n)
                if it < IT - 1:
                    A_n, S_n = derive_state(so_n, b, new_negs)
                    new_AS.append((A_n, S_n))
                    bcast_A(A_n, b)
            so = new_so
            if it < IT - 1:
                AS = new_AS
                negs = new_negs

        # output: so (cm) -> row-major [BL, N]
        for b in range(BL):
            po = pstr_p.tile([NC, 128], F32, tag="tr", name="tr")
            nc.tensor.transpose(po[:], so[b], ident)
            nc.vector.tensor_copy(at_sb[b][:], po[:])
            nc.sync.dma_start(d_out[b : b + 1, :], at_sb[b][:])

    return nc


_CACHE = {}


def _get_nc():
    if "nc" not in _CACHE:
        nc = bacc.Bacc("TRN2", target_bir_lowering=False)
        build(nc)
        nc.compile()
        _CACHE["nc"] = nc
    return _CACHE["nc"]


def _cm(row, ncnk):  # [N] -> [128, ncnk] column-major
    return row.reshape(ncnk, 128).T


def kernel(soft_input, H, labels, w):
    del labels  # unused by the reference computation
    soft_input = np.asarray(soft_input, dtype=np.float32)
    H = np.asarray(H)
    w = np.asarray(w, dtype=np.float32)
    NC = N // 128

    norm = np.log1p(np.exp(np.float64(w[0]))).astype(np.float32)
    f8 = mybir.dt.np(F8)
    bf = mybir.dt.np(BF16)
    h_bf = H.astype(bf)
    h_big = ((1 - H) * NEG_BACK).astype(mybir.dt.np(F8E5))
    h_t = np.ascontiguousarray(H.T).astype(f8)
    misc_in = np.concatenate(
        [np.eye(128, dtype=np.float32), np.full((128, 1), norm, dtype=np.float32)], axis=1
    )

    in_maps = []
    for core in range(N_CORES):
        rows = soft_input[BL * core : BL * (core + 1)]
        si_cm = np.concatenate([_cm(rows[b], NC) for b in range(BL)], axis=1)
        abc0 = np.concatenate(
            [np.broadcast_to(np.abs(rows[b])[None, :], (128, N)) for b in range(BL)], axis=1
        )
        in_maps.append(
            {
                "si_cm": np.ascontiguousarray(si_cm, dtype=np.float32),
                "h_bf": h_bf,
                "h_big": h_big,
                "h_t": h_t,
                "misc_in": misc_in,
                "abc0": np.ascontiguousarray(abc0, dtype=np.float32),
            }
        )

    nc = _get_nc()
    res = bass_utils.run_bass_kernel_spmd(nc, in_maps, core_ids=list(range(N_CORES)))
    out = np.concatenate([r["out"] for r in res.results], axis=0)
    return out.astype(np.float32)



# revision 6
# speedup vs baseline: 1.6707x; 1.6707x over previous
"""Trainium2 Bass kernel: normalized min-sum LDPC decoder (nn_Decoding_model).

Sharding: pure batch data-parallelism. B=16 rows split across 8 NeuronCores
(2 rows/core); H-derived tables are replicated per core.

Sparse-compact algorithm (per core, per iteration, BL=2 batch rows):
  pack    : per column n, s[n] = -(ql[n]*2048 + n), ql = log-quantized |so_n|
            (12.7-bit log scale). Ordering of s == ordering of (|so|, n), and
            the argmin column index rides in the low 11 bits.
  bcast   : s row-broadcast to [128, N] via DRAM round trip (as in min-sum
            baseline) -> bc[b].
  gather  : ONE gpsimd.ap_gather per b pulls, for every 16-partition group,
            the concatenated edge lists of its 8 chunk-rows: out [128, 8*SEC].
            Partition p's section mc holds group (p//16)'s edges; p's own
            edges sit in a fixed subrange.
  mask+max: masked = gath - maskc (maskc = +2^26 off-range), then DVE max8
            per section -> v0, v1 = two largest s = two smallest (|so|, n).
  extract : V = -v; n01 = mod(V, 2048); ql = (V - n01)/2048;
            m1, m2 = exp(ql*ln2/K - OFF*ln2) (Act engine).
  parity  : PE matmul H^T(fp8) x (so<0) -> row sign rs (exact counts).
  msgs    : t1 = rs*m1 (2-way bf16 split), t2 = rs*(m2-m1) (2-way fp16 split).
  onehot  : oh = is_equal(nrow, n*) fp16 [128, N] (DVE 4x mode) -- exact
            argmin onehot since the packing is injective in n.
  matmul  : main: sum_m H(bf16)*t1 and corr: sum_m oh(fp16)*t2 accumulate
            into the SAME PSUM tile (start/stop chaining).
  combine : so_new = si + softplus(w)*sign(so)*(main+corr).

Layout "cm" = column-major [128, NC]: n = c*128 + p.
"""

from contextlib import ExitStack

import numpy as np

import concourse.bass as bass
import concourse.mybir as mybir
import concourse.tile as tile
from concourse import bacc, bass_utils

F32 = mybir.dt.float32
F16 = mybir.dt.float16
BF16 = mybir.dt.bfloat16
F8 = mybir.dt.float8e4
I16 = mybir.dt.int16
I32 = mybir.dt.int32
OP = mybir.AluOpType
AF = mybir.ActivationFunctionType

B, M, N, IT = 16, 1024, 2048, 5
N_CORES = 8
BL = B // N_CORES
MC = M // 128
NC = N // 128

SEC = 420            # per-(mc, group) concat-list width (max group degree sum is 417)
NIDX = MC * SEC      # mega-gather width per b
KQ = 409.0           # log-quant steps per octave
OFF = 14.0           # |so| clamped below at 2^-OFF
LN2 = 0.6931471805599453
QSC = KQ / LN2       # ln(a) -> log2(a)*K
QBI = OFF * KQ + 0.5 # offset + round-to-nearest bias
ISC = LN2 / KQ       # ql -> exp scale
IBI = -OFF * LN2
QMAX = 8191.0
MASKV = float(2 ** 26)


def build(nc: bass.Bass, pool_masks=12, oh_bufs=10, gath_bufs=2, msk_bufs=6):
    d_si = nc.dram_tensor("si_cm", [128, NC * BL], F32, kind="ExternalInput").ap()
    d_h = nc.dram_tensor("h_bf", [M, N], BF16, kind="ExternalInput").ap()
    d_ht = nc.dram_tensor("h_t", [N, M], F8, kind="ExternalInput").ap()
    d_misc = nc.dram_tensor("misc_in", [128, 131 + NC], F32, kind="ExternalInput").ap()
    d_mask = nc.dram_tensor("maskc", [128, NIDX], BF16, kind="ExternalInput").ap()
    d_gidx = nc.dram_tensor("gidx", [128, NIDX // 16], I16, kind="ExternalInput").ap()
    d_nrow = nc.dram_tensor("nrow", [128, N], F16, kind="ExternalInput").ap()
    d_pk0 = nc.dram_tensor("pk0", [128, N * BL], F32, kind="ExternalInput").ap()
    d_out = nc.dram_tensor("out", [BL, N], F32, kind="ExternalOutput").ap()

    with tile.TileContext(nc) as tc, ExitStack() as ctx:
        const = ctx.enter_context(tc.tile_pool(name="const", bufs=1))
        state_p = ctx.enter_context(tc.tile_pool(name="state", bufs=2))
        gath_p = ctx.enter_context(tc.tile_pool(name="gath", bufs=gath_bufs))
        bc_p = ctx.enter_context(tc.tile_pool(name="bc", bufs=2))
        oh_p = ctx.enter_context(tc.tile_pool(name="oh", bufs=oh_bufs))
        msk_p = ctx.enter_context(tc.tile_pool(name="msk", bufs=msk_bufs))
        psum_p = ctx.enter_context(tc.tile_pool(name="ps", bufs=1, space="PSUM"))
        pstr_p = ctx.enter_context(tc.tile_pool(name="pstr", bufs=2, space="PSUM"))
        dram_p = ctx.enter_context(tc.tile_pool(name="dram", bufs=1, space="DRAM"))

        # ---- persistent loads ----
        misc = const.tile([128, 131 + NC], F32)
        ident = misc[:, 0:128]
        t_norm = misc[:, 128:129]
        t_ibi = misc[:, 129:130]   # -OFF*ln2 (Act Exp bias)
        t_isc = misc[:, 130:131]   # ln2/KQ/2048 (Act Exp scale)
        t_iota = misc[:, 131 : 131 + NC]  # iota_cm: n = c*128 + p
        nc.sync.dma_start(misc[:], d_misc)
        # dummy matmul so the PE observes misc's DMA sem before any transpose
        # (transpose-mode matmuls only support a single sync wait)
        pdum = pstr_p.tile([1, 1], F32, tag="dum", name="dum", bufs=1)
        nc.tensor.matmul(pdum[:], lhsT=ident[:, 0:1], rhs=ident[:, 0:1], start=True, stop=True)

        t_gidx = const.tile([128, NIDX // 16], I16)
        nc.sync.dma_start(t_gidx[:], d_gidx)
        t_mask = const.tile([128, NIDX], BF16)
        nc.sync.dma_start(t_mask[:], d_mask)
        t_nrow = const.tile([128, N], F16)
        nc.sync.dma_start(t_nrow[:], d_nrow)
        t_ht = const.tile([128, NC * M], F8)
        ht_r = d_ht.rearrange("(c p) m -> c p m", p=128)
        for c in range(NC):
            nc.sync.dma_start(t_ht[:, c * M : (c + 1) * M], ht_r[c])
        t_h = const.tile([128, MC * N], BF16)
        h_r = d_h.rearrange("(mc p) n -> mc p n", p=128)
        for mc in range(MC):
            nc.sync.dma_start(t_h[:, mc * N : (mc + 1) * N], h_r[mc])

        t_si = const.tile([128, NC * BL], F32)
        nc.sync.dma_start(t_si[:], d_si)
        si = [t_si[:, NC * b : NC * (b + 1)] for b in range(BL)]

        at_sb = [const.tile([NC, 128], F32, tag=f"atsb{b}", name=f"atsb{b}") for b in range(BL)]
        d_arow = dram_p.tile([BL, N], F32, name="d_arow")

        def derive_state(so_ap, b, negs):
            """negs (fp8 col 2c+b) = (so<0); S = sign(so) in {-1, +1}."""
            st = state_p.tile([128, NC], F32, tag=f"sgn{b}", name=f"sgn{b}")
            nc.vector.tensor_scalar(out=st, in0=so_ap, scalar1=0.0, scalar2=2.0, op0=OP.is_ge, op1=OP.mult)
            nc.vector.tensor_scalar(out=st, in0=st, scalar1=-1.0, scalar2=None, op0=OP.add)
            nc.vector.tensor_scalar(
                out=negs[:].rearrange("p (c two) -> p c two", two=2)[:, :, b : b + 1],
                in0=so_ap.unsqueeze(2),
                scalar1=0.0, scalar2=None, op0=OP.is_lt,
            )
            return st

        def pack_bcast(so_ap, b):
            """Quantize-pack so (cm) and broadcast s row to bc[b] [128, N]."""
            pk = state_p.tile([128, 3 * NC], F32, tag=f"pk{b}", name=f"pk{b}")
            aa, ff, fm = pk[:, 0:NC], pk[:, NC : 2 * NC], pk[:, 2 * NC : 3 * NC]
            nc.vector.tensor_scalar(
                out=aa.bitcast(I32), in0=so_ap.bitcast(I32),
                scalar1=0x7FFFFFFF, scalar2=None, op0=OP.bitwise_and,
            )
            nc.vector.tensor_scalar(out=aa, in0=aa, scalar1=2.0 ** (-OFF), scalar2=None, op0=OP.max)
            nc.scalar.activation(ff, aa, AF.Ln)
            nc.vector.tensor_scalar(out=ff, in0=ff, scalar1=QSC, scalar2=QBI, op0=OP.mult, op1=OP.add)
            nc.vector.tensor_scalar(out=ff, in0=ff, scalar1=0.0, scalar2=QMAX, op0=OP.max, op1=OP.min)
            qi = fm.bitcast(I32)
            qi2 = aa.bitcast(I32)
            nc.vector.tensor_copy(qi, ff)  # ql = round (i32 convert)
            nc.vector.tensor_scalar(out=qi2, in0=qi, scalar1=-2048, scalar2=None, op0=OP.mult)
            nc.vector.tensor_copy(ff, qi2)
            nc.vector.tensor_tensor(out=ff, in0=ff, in1=t_iota, op=OP.subtract)  # -(ql*2048+n)
            pt = pstr_p.tile([NC, 128], F32, tag="tr", name="tr")
            nc.tensor.transpose(pt[:], ff, ident)
            nc.vector.tensor_copy(at_sb[b][:], pt[:])
            nc.sync.dma_start(d_arow[b : b + 1, :], at_sb[b][:])
            bc = bc_p.tile([128, N], F32, tag=f"bc{b}", name=f"bc{b}")
            for q in range(4):
                nc.sync.dma_start(
                    bc[:, q * (N // 4) : (q + 1) * (N // 4)],
                    d_arow[b : b + 1, q * (N // 4) : (q + 1) * (N // 4)].to_broadcast([128, N // 4]),
                )
            return bc

        # ---- init state ----
        so = [si[b] for b in range(BL)]
        negs = state_p.tile([128, 2 * NC], F8, tag="negs", name="negs")
        S = [derive_state(so[b], b, negs) for b in range(BL)]
        bc = [None] * BL
        for b in range(BL):
            bc[b] = bc_p.tile([128, N], F32, tag=f"bc{b}", name=f"bc{b}")
            for q in range(2):
                nc.sync.dma_start(
                    bc[b][:, q * (N // 2) : (q + 1) * (N // 2)],
                    d_pk0[:, N * b + q * (N // 2) : N * b + (q + 1) * (N // 2)],
                )

        for it in range(IT):
            # parity: pp[m-part, 2*mc+b] = sum_n H^T * (so<0)  (PE, fp8 exact)
            pp = psum_p.tile([128, 2 * MC], F32, tag="pp", name="pp")
            for mc in range(MC):
                for c in range(NC):
                    nc.tensor.matmul(
                        pp[:, 2 * mc : 2 * mc + 2],
                        lhsT=t_ht[:, c * M + 128 * mc : c * M + 128 * (mc + 1)],
                        rhs=negs[:, 2 * c : 2 * c + 2],
                        start=(c == 0),
                        stop=(c == NC - 1),
                    )
            ri = state_p.tile([128, 2 * MC], I32, tag="ri", name="ri")
            rs = state_p.tile([128, 2 * MC], F32, tag="rs", name="rs")
            nc.vector.tensor_copy(ri[:], pp[:])  # exact: P is integer-valued
            nc.vector.tensor_scalar(out=ri[:], in0=ri[:], scalar1=1, scalar2=None, op0=OP.bitwise_and)
            nc.vector.tensor_copy(rs[:], ri[:])
            nc.vector.tensor_scalar(out=rs[:], in0=rs[:], scalar1=-2.0, scalar2=1.0, op0=OP.mult, op1=OP.add)
            rs_v = rs[:].rearrange("p (c two) -> p c two", two=2)

            new_negs = state_p.tile([128, 2 * NC], F8, tag="negs", name="negs") if it < IT - 1 else None
            new_so, new_S = [], []
            for b in range(BL):
                gath = gath_p.tile([128, NIDX], F32, tag=f"g{b}", name=f"g{b}")
                nc.gpsimd.ap_gather(
                    gath[:], bc[b][:], t_gidx[:], channels=128, num_elems=N, d=1, num_idxs=NIDX
                )
                vv = state_p.tile([128, 8 * MC], F32, tag=f"vv{b}", name=f"vv{b}")
                for mc in range(MC):
                    msk = msk_p.tile([128, SEC], F32, tag="mk", name="mk")
                    eng = nc.gpsimd if mc < pool_masks else nc.vector
                    eng.tensor_tensor(
                        out=msk[:],
                        in0=gath[:, mc * SEC : (mc + 1) * SEC],
                        in1=t_mask[:, mc * SEC : (mc + 1) * SEC],
                        op=OP.subtract,
                    )
                    nc.vector.max(out=vv[:, 8 * mc : 8 * mc + 8], in_=msk[:])

                # extract: V = -v01; n01 = mod(V, 2048); ql = (V-n01)/2048; m = exp(...)
                ex = state_p.tile([128, 10 * MC], F32, tag=f"ex{b}", name=f"ex{b}")
                Vt = ex[:, 0 : 2 * MC].rearrange("p (c k) -> p c k", k=2)
                n01 = ex[:, 2 * MC : 4 * MC].rearrange("p (c k) -> p c k", k=2)
                m12 = ex[:, 4 * MC : 6 * MC].rearrange("p (c k) -> p c k", k=2)
                vi = ex[:, 6 * MC : 8 * MC].bitcast(I32).rearrange("p (c k) -> p c k", k=2)
                ti = ex[:, 8 * MC : 10 * MC].bitcast(I32).rearrange("p (c k) -> p c k", k=2)
                vv8 = vv[:].rearrange("p (c k) -> p c k", k=8)
                nc.vector.tensor_scalar(out=Vt, in0=vv8[:, :, 0:2], scalar1=-1.0, scalar2=None, op0=OP.mult)
                nc.vector.tensor_copy(vi, Vt)  # exact int convert
                nc.vector.tensor_scalar(out=ti, in0=vi, scalar1=2047, scalar2=None, op0=OP.bitwise_and)
                nc.vector.tensor_copy(n01, ti)
                nc.vector.tensor_scalar(out=vi, in0=vi, scalar1=~2047, scalar2=None, op0=OP.bitwise_and)
                nc.vector.tensor_copy(Vt, vi)
                nc.scalar.activation(
                    m12.rearrange("p c k -> p (c k)"), Vt.rearrange("p c k -> p (c k)"),
                    AF.Exp, bias=t_ibi, scale=t_isc,
                )

                # messages + 2-way splits
                tm = state_p.tile([128, 4 * MC], F32, tag=f"tm{b}", name=f"tm{b}")
                t1 = tm[:, 0:MC].unsqueeze(2)
                t2 = tm[:, MC : 2 * MC].unsqueeze(2)
                tr1 = tm[:, 2 * MC : 3 * MC].unsqueeze(2)
                tr2 = tm[:, 3 * MC : 4 * MC].unsqueeze(2)
                smb = state_p.tile([128, 2 * MC], BF16, tag=f"smb{b}", name=f"smb{b}")
                smf = state_p.tile([128, 2 * MC], F16, tag=f"smf{b}", name=f"smf{b}")
                t1h = smb[:].rearrange("p (c k) -> p c k", k=2)[:, :, 0:1]
                t1l = smb[:].rearrange("p (c k) -> p c k", k=2)[:, :, 1:2]
                t2h = smf[:].rearrange("p (c k) -> p c k", k=2)[:, :, 0:1]
                t2l = smf[:].rearrange("p (c k) -> p c k", k=2)[:, :, 1:2]
                nc.vector.tensor_tensor(out=t1, in0=m12[:, :, 0:1], in1=rs_v[:, :, b : b + 1], op=OP.mult)
                nc.vector.tensor_tensor(out=t2, in0=m12[:, :, 1:2], in1=m12[:, :, 0:1], op=OP.subtract)
                nc.vector.tensor_tensor(out=t2, in0=t2, in1=rs_v[:, :, b : b + 1], op=OP.mult)
                nc.vector.tensor_copy(t1h, t1)
                nc.vector.tensor_tensor(out=tr1, in0=t1, in1=t1h, op=OP.subtract)
                nc.vector.tensor_copy(t1l, tr1)
                nc.vector.tensor_copy(t2h, t2)
                nc.vector.tensor_tensor(out=tr2, in0=t2, in1=t2h, op=OP.subtract)
                nc.vector.tensor_copy(t2l, tr2)

                # onehot per mc (fp16, DVE 4x): oh = (nrow == n*)
                ohs = {}
                for mc in range(MC):
                    oh = oh_p.tile([128, N], F16, tag="oh", name="oh")
                    ohs[mc] = oh
                    nc.vector.tensor_scalar(
                        out=oh[:], in0=t_nrow[:], scalar1=n01[:, mc, 0:1], scalar2=None, op0=OP.is_equal
                    )

                # main + corr accumulate into one PSUM tile per b
                pm = psum_p.tile([128, 2 * NC], F32, tag=f"pm{b}", name=f"pm{b}")
                smb_v = smb[:].rearrange("p (c k) -> p c k", k=2)
                smf_v = smf[:].rearrange("p (c k) -> p c k", k=2)
                for c in range(NC):
                    for mc in range(MC):
                        nc.tensor.matmul(
                            pm[:, 2 * c : 2 * c + 2],
                            lhsT=t_h[:, mc * N + 128 * c : mc * N + 128 * (c + 1)],
                            rhs=smb_v[:, mc : mc + 1, 0:2],
                            start=(mc == 0),
                            stop=False,
                        )
                    for mc in range(MC):
                        nc.tensor.matmul(
                            pm[:, 2 * c : 2 * c + 2],
                            lhsT=ohs[mc][:, 128 * c : 128 * (c + 1)],
                            rhs=smf_v[:, mc : mc + 1, 0:2],
                            start=False,
                            stop=(mc == MC - 1),
                        )

                # combine
                st2 = state_p.tile([128, 2 * NC], F32, tag=f"cmb{b}", name=f"cmb{b}")
                crr, so_n = st2[:, 0:NC], st2[:, NC : 2 * NC]
                pm_v = pm[:].rearrange("p (c k) -> p c k", k=2)
                nc.vector.tensor_copy(crr.unsqueeze(2), pm_v[:, :, 0:1])
                nc.vector.tensor_tensor(out=crr.unsqueeze(2), in0=crr.unsqueeze(2), in1=pm_v[:, :, 1:2], op=OP.add)
                nc.vector.scalar_tensor_tensor(
                    out=crr, in0=crr, scalar=t_norm, in1=S[b], op0=OP.mult, op1=OP.mult
                )
                nc.vector.tensor_tensor(out=so_n, in0=si[b], in1=crr, op=OP.add)
                new_so.append(so_n)
                if it < IT - 1:
                    new_S.append(derive_state(so_n, b, new_negs))
                    bc[b] = pack_bcast(so_n, b)
            so = new_so
            if it < IT - 1:
                S = new_S
                negs = new_negs

        # output: so (cm) -> row-major [BL, N]
        for b in range(BL):
            po = pstr_p.tile([NC, 128], F32, tag="tr", name="tr")
            nc.tensor.transpose(po[:], so[b], ident)
            nc.vector.tensor_copy(at_sb[b][:], po[:])
            nc.sync.dma_start(d_out[b : b + 1, :], at_sb[b][:])

    return nc


_CACHE = {}


def _get_nc():
    if "nc" not in _CACHE:
        nc = bacc.Bacc("TRN2", target_bir_lowering=False)
        build(nc)
        nc.compile()
        _CACHE["nc"] = nc
    return _CACHE["nc"]


def _cm(row, ncnk):  # [N] -> [128, ncnk] column-major
    return row.reshape(ncnk, 128).T


def _pack_rows(rows):
    """rows [BL, N] f32 -> packed negated rows [BL, N] f32."""
    a = np.abs(rows.astype(np.float64))
    ql = np.clip(np.floor((np.log2(np.maximum(a, 2.0 ** (-OFF))) + OFF) * KQ + 0.5), 0.0, QMAX)
    return (-(ql * 2048.0 + np.arange(N)[None, :])).astype(np.float32)


def _tables(H):
    """Gather index lists, own-range mask, per-H constants."""
    gidx = np.zeros((128, NIDX // 16), np.int16)
    maskc = np.full((128, NIDX), MASKV, np.float32)
    for g in range(8):
        glist = np.zeros(NIDX, np.int16)
        for mc in range(MC):
            lo = mc * SEC
            pos = 0
            for r in range(16):
                m = mc * 128 + 16 * g + r
                e = np.nonzero(H[m])[0]
                assert pos + len(e) <= SEC, (m, pos, len(e))
                glist[lo + pos : lo + pos + len(e)] = e
                maskc[16 * g + r, lo + pos : lo + pos + len(e)] = 0.0
                pos += len(e)
        for i in range(NIDX):
            gidx[16 * g + i % 16, i // 16] = glist[i]
    return gidx, maskc


def kernel(soft_input, H, labels, w):
    del labels  # unused by the reference computation
    soft_input = np.asarray(soft_input, dtype=np.float32)
    H = np.asarray(H)
    w = np.asarray(w, dtype=np.float32)

    norm = np.log1p(np.exp(np.float64(w[0]))).astype(np.float32)
    bf = mybir.dt.np(BF16)
    f16 = mybir.dt.np(F16)
    f8 = mybir.dt.np(F8)
    h_bf = H.astype(bf)
    h_t = np.ascontiguousarray(H.T).astype(f8)
    gidx, maskc = _tables(H)
    maskc_bf = maskc.astype(bf)
    nrow = np.tile(np.arange(N, dtype=np.float32).astype(f16)[None, :], (128, 1))
    iota_cm = np.arange(N, dtype=np.float32).reshape(NC, 128).T
    misc_in = np.concatenate(
        [
            np.eye(128, dtype=np.float32),
            np.full((128, 1), norm, dtype=np.float32),
            np.full((128, 1), IBI, dtype=np.float32),
            np.full((128, 1), ISC / 2048.0, dtype=np.float32),
            iota_cm,
        ],
        axis=1,
    )

    in_maps = []
    for core in range(N_CORES):
        rows = soft_input[BL * core : BL * (core + 1)]
        si_cm = np.concatenate([_cm(rows[b], NC) for b in range(BL)], axis=1)
        pk = _pack_rows(rows)
        pk0 = np.concatenate(
            [np.broadcast_to(pk[b][None, :], (128, N)) for b in range(BL)], axis=1
        )
        in_maps.append(
            {
                "si_cm": np.ascontiguousarray(si_cm, dtype=np.float32),
                "h_bf": h_bf,
                "h_t": h_t,
                "misc_in": misc_in,
                "maskc": maskc_bf,
                "gidx": gidx,
                "nrow": nrow,
                "pk0": np.ascontiguousarray(pk0, dtype=np.float32),
            }
        )

    nc = _get_nc()
    res = bass_utils.run_bass_kernel_spmd(nc, in_maps, core_ids=list(range(N_CORES)))
    out = np.concatenate([r["out"] for r in res.results], axis=0)
    return out.astype(np.float32)


# revision 7
# speedup vs baseline: 1.6864x; 1.0094x over previous
"""Trainium2 Bass kernel: normalized min-sum LDPC decoder (nn_Decoding_model).

Sharding: pure batch data-parallelism. B=16 rows split across 8 NeuronCores
(2 rows/core); H-derived tables are replicated per core.

Sparse-compact algorithm (per core, per iteration, BL=2 batch rows):
  pack    : per column n, s[n] = -(ql[n]*2048 + n), ql = log-quantized |so_n|
            (12.7-bit log scale). Ordering of s == ordering of (|so|, n), and
            the argmin column index rides in the low 11 bits.
  bcast   : s row-broadcast to [128, N] via DRAM round trip (as in min-sum
            baseline) -> bc[b].
  gather  : ONE gpsimd.ap_gather per b pulls, for every 16-partition group,
            the concatenated edge lists of its 8 chunk-rows: out [128, 8*SEC].
            Partition p's section mc holds group (p//16)'s edges; p's own
            edges sit in a fixed subrange.
  mask+max: masked = gath - maskc (maskc = +2^26 off-range), then DVE max8
            per section -> v0, v1 = two largest s = two smallest (|so|, n).
  extract : V = -v; n01 = mod(V, 2048); ql = (V - n01)/2048;
            m1, m2 = exp(ql*ln2/K - OFF*ln2) (Act engine).
  parity  : PE matmul H^T(fp8) x (so<0) -> row sign rs (exact counts).
  msgs    : t1 = rs*m1 (2-way bf16 split), t2 = rs*(m2-m1) (2-way fp16 split).
  onehot  : oh = is_equal(nrow, n*) fp16 [128, N] (DVE 4x mode) -- exact
            argmin onehot since the packing is injective in n.
  matmul  : main: sum_m H(bf16)*t1 and corr: sum_m oh(fp16)*t2 accumulate
            into the SAME PSUM tile (start/stop chaining).
  combine : so_new = si + softplus(w)*sign(so)*(main+corr).

Layout "cm" = column-major [128, NC]: n = c*128 + p.
"""

from contextlib import ExitStack

import numpy as np

import concourse.bass as bass
import concourse.mybir as mybir
import concourse.tile as tile
from concourse import bacc, bass_utils

F32 = mybir.dt.float32
F16 = mybir.dt.float16
BF16 = mybir.dt.bfloat16
F8 = mybir.dt.float8e4
I16 = mybir.dt.int16
I32 = mybir.dt.int32
OP = mybir.AluOpType
AF = mybir.ActivationFunctionType

B, M, N, IT = 16, 1024, 2048, 5
N_CORES = 8
BL = B // N_CORES
MC = M // 128
NC = N // 128

SEC = 420            # per-(mc, group) concat-list width (max group degree sum is 417)
NIDX = MC * SEC      # mega-gather width per b
MASKV = float(2 ** 26)
# float-bits log quantization: clamp |so| to [2^-10, 2^6), truncate mantissa
# to 9 bits (512 steps/octave); sort key = (bits-B0)>>3 + n fits 24 bits.
B0I = 117 << 23          # bits(2^-10)
B0F = float(B0I)
CLO = 2.0 ** (-10)
CHI = 63.99999
QMASK = ~16383           # keep exponent + top 9 mantissa bits


def build(nc: bass.Bass, pool_masks=6, oh_bufs=10, gath_bufs=2, msk_bufs=6):
    d_si = nc.dram_tensor("si_cm", [128, NC * BL], F32, kind="ExternalInput").ap()
    d_h = nc.dram_tensor("h_bf", [M, N], BF16, kind="ExternalInput").ap()
    d_ht = nc.dram_tensor("h_t", [N, M], F8, kind="ExternalInput").ap()
    d_misc = nc.dram_tensor("misc_in", [128, 129 + NC], F32, kind="ExternalInput").ap()
    d_mask = nc.dram_tensor("maskc", [128, NIDX], BF16, kind="ExternalInput").ap()
    d_gidx = nc.dram_tensor("gidx", [128, NIDX // 16], I16, kind="ExternalInput").ap()
    d_nrow = nc.dram_tensor("nrow", [128, N], F16, kind="ExternalInput").ap()
    d_pk0 = nc.dram_tensor("pk0", [128, N * BL], F32, kind="ExternalInput").ap()
    d_out = nc.dram_tensor("out", [BL, N], F32, kind="ExternalOutput").ap()

    with tile.TileContext(nc) as tc, ExitStack() as ctx:
        const = ctx.enter_context(tc.tile_pool(name="const", bufs=1))
        state_p = ctx.enter_context(tc.tile_pool(name="state", bufs=2))
        gath_p = ctx.enter_context(tc.tile_pool(name="gath", bufs=gath_bufs))
        bc_p = ctx.enter_context(tc.tile_pool(name="bc", bufs=2))
        oh_p = ctx.enter_context(tc.tile_pool(name="oh", bufs=oh_bufs))
        msk_p = ctx.enter_context(tc.tile_pool(name="msk", bufs=msk_bufs))
        psum_p = ctx.enter_context(tc.tile_pool(name="ps", bufs=1, space="PSUM"))
        pstr_p = ctx.enter_context(tc.tile_pool(name="pstr", bufs=2, space="PSUM"))
        dram_p = ctx.enter_context(tc.tile_pool(name="dram", bufs=1, space="DRAM"))

        # ---- persistent loads ----
        misc = const.tile([128, 129 + NC], F32)
        ident = misc[:, 0:128]
        t_norm = misc[:, 128:129]
        t_iota = misc[:, 129 : 129 + NC]  # iota_cm: n = c*128 + p
        nc.sync.dma_start(misc[:], d_misc)
        # dummy matmul so the PE observes misc's DMA sem before any transpose
        # (transpose-mode matmuls only support a single sync wait)
        pdum = pstr_p.tile([1, 1], F32, tag="dum", name="dum", bufs=1)
        nc.tensor.matmul(pdum[:], lhsT=ident[:, 0:1], rhs=ident[:, 0:1], start=True, stop=True)

        t_gidx = const.tile([128, NIDX // 16], I16)
        nc.sync.dma_start(t_gidx[:], d_gidx)
        t_mask = const.tile([128, NIDX], BF16)
        nc.sync.dma_start(t_mask[:], d_mask)
        t_nrow = const.tile([128, N], F16)
        nc.sync.dma_start(t_nrow[:], d_nrow)
        t_ht = const.tile([128, NC * M], F8)
        ht_r = d_ht.rearrange("(c p) m -> c p m", p=128)
        for c in range(NC):
            nc.sync.dma_start(t_ht[:, c * M : (c + 1) * M], ht_r[c])
        t_h = const.tile([128, MC * N], BF16)
        h_r = d_h.rearrange("(mc p) n -> mc p n", p=128)
        for mc in range(MC):
            nc.sync.dma_start(t_h[:, mc * N : (mc + 1) * N], h_r[mc])

        t_si = const.tile([128, NC * BL], F32)
        nc.sync.dma_start(t_si[:], d_si)
        si = [t_si[:, NC * b : NC * (b + 1)] for b in range(BL)]

        at_sb = [const.tile([NC, 128], F32, tag=f"atsb{b}", name=f"atsb{b}") for b in range(BL)]
        d_arow = dram_p.tile([BL, N], F32, name="d_arow")

        def derive_state(so_ap, b, negs):
            """negs (fp8 col 2c+b) = (so<0); S = sign(so) in {-1, +1}."""
            st = state_p.tile([128, NC], F32, tag=f"sgn{b}", name=f"sgn{b}")
            nc.vector.tensor_scalar(out=st, in0=so_ap, scalar1=0.0, scalar2=2.0, op0=OP.is_ge, op1=OP.mult)
            nc.vector.tensor_scalar(out=st, in0=st, scalar1=-1.0, scalar2=None, op0=OP.add)
            nc.vector.tensor_scalar(
                out=negs[:].rearrange("p (c two) -> p c two", two=2)[:, :, b : b + 1],
                in0=so_ap.unsqueeze(2),
                scalar1=0.0, scalar2=None, op0=OP.is_lt,
            )
            return st

        def pack_bcast(so_ap, b):
            """Quantize-pack so (cm) and broadcast s row to bc[b] [128, N]."""
            pk = state_p.tile([128, 3 * NC], F32, tag=f"pk{b}", name=f"pk{b}")
            aa, ff, fm = pk[:, 0:NC], pk[:, NC : 2 * NC], pk[:, 2 * NC : 3 * NC]
            nc.vector.tensor_scalar(
                out=aa.bitcast(I32), in0=so_ap.bitcast(I32),
                scalar1=0x7FFFFFFF, scalar2=None, op0=OP.bitwise_and,
            )
            nc.vector.tensor_scalar(out=aa, in0=aa, scalar1=CLO, scalar2=CHI, op0=OP.max, op1=OP.min)
            nc.vector.tensor_scalar(
                out=fm.bitcast(I32), in0=aa.bitcast(I32), scalar1=QMASK, scalar2=None, op0=OP.bitwise_and
            )
            nc.vector.tensor_copy(ff, fm.bitcast(I32))  # exact: low 14 bits are zero
            nc.vector.tensor_scalar(out=ff, in0=ff, scalar1=-B0F, scalar2=-0.125, op0=OP.add, op1=OP.mult)
            nc.vector.tensor_tensor(out=ff, in0=ff, in1=t_iota, op=OP.subtract)  # -((bits-B0)/8 + n)
            pt = pstr_p.tile([NC, 128], F32, tag="tr", name="tr")
            nc.tensor.transpose(pt[:], ff, ident)
            nc.vector.tensor_copy(at_sb[b][:], pt[:])
            nc.sync.dma_start(d_arow[b : b + 1, :], at_sb[b][:])
            bc = bc_p.tile([128, N], F32, tag=f"bc{b}", name=f"bc{b}")
            for q in range(4):
                nc.sync.dma_start(
                    bc[:, q * (N // 4) : (q + 1) * (N // 4)],
                    d_arow[b : b + 1, q * (N // 4) : (q + 1) * (N // 4)].to_broadcast([128, N // 4]),
                )
            return bc

        # ---- init state ----
        so = [si[b] for b in range(BL)]
        negs = state_p.tile([128, 2 * NC], F8, tag="negs", name="negs")
        S = [derive_state(so[b], b, negs) for b in range(BL)]
        bc = [None] * BL
        for b in range(BL):
            bc[b] = bc_p.tile([128, N], F32, tag=f"bc{b}", name=f"bc{b}")
            for q in range(2):
                nc.sync.dma_start(
                    bc[b][:, q * (N // 2) : (q + 1) * (N // 2)],
                    d_pk0[:, N * b + q * (N // 2) : N * b + (q + 1) * (N // 2)],
                )

        for it in range(IT):
            # parity: pp[m-part, 2*mc+b] = sum_n H^T * (so<0)  (PE, fp8 exact)
            pp = psum_p.tile([128, 2 * MC], F32, tag="pp", name="pp")
            for mc in range(MC):
                for c in range(NC):
                    nc.tensor.matmul(
                        pp[:, 2 * mc : 2 * mc + 2],
                        lhsT=t_ht[:, c * M + 128 * mc : c * M + 128 * (mc + 1)],
                        rhs=negs[:, 2 * c : 2 * c + 2],
                        start=(c == 0),
                        stop=(c == NC - 1),
                    )
            ri = state_p.tile([128, 2 * MC], I32, tag="ri", name="ri")
            rs = state_p.tile([128, 2 * MC], F32, tag="rs", name="rs")
            nc.vector.tensor_copy(ri[:], pp[:])  # exact: P is integer-valued
            nc.vector.tensor_scalar(out=ri[:], in0=ri[:], scalar1=1, scalar2=None, op0=OP.bitwise_and)
            nc.vector.tensor_copy(rs[:], ri[:])
            nc.vector.tensor_scalar(out=rs[:], in0=rs[:], scalar1=-2.0, scalar2=1.0, op0=OP.mult, op1=OP.add)
            rs_v = rs[:].rearrange("p (c two) -> p c two", two=2)

            new_negs = state_p.tile([128, 2 * NC], F8, tag="negs", name="negs") if it < IT - 1 else None
            new_so, new_S = [], []
            for b in range(BL):
                gath = gath_p.tile([128, NIDX], F32, tag=f"g{b}", name=f"g{b}")
                nc.gpsimd.ap_gather(
                    gath[:], bc[b][:], t_gidx[:], channels=128, num_elems=N, d=1, num_idxs=NIDX
                )
                vv = state_p.tile([128, 8 * MC], F32, tag=f"vv{b}", name=f"vv{b}")
                for mc in range(MC):
                    msk = msk_p.tile([128, SEC], F32, tag="mk", name="mk")
                    eng = nc.gpsimd if mc < pool_masks else nc.vector
                    eng.tensor_tensor(
                        out=msk[:],
                        in0=gath[:, mc * SEC : (mc + 1) * SEC],
                        in1=t_mask[:, mc * SEC : (mc + 1) * SEC],
                        op=OP.subtract,
                    )
                    nc.vector.max(out=vv[:, 8 * mc : 8 * mc + 8], in_=msk[:])

                # extract: V = -v01; n01 = mod(V, 2048); ql = (V-n01)/2048; m = exp(...)
                ex = state_p.tile([128, 10 * MC], F32, tag=f"ex{b}", name=f"ex{b}")
                Vt = ex[:, 0 : 2 * MC].rearrange("p (c k) -> p c k", k=2)
                n01 = ex[:, 2 * MC : 4 * MC].rearrange("p (c k) -> p c k", k=2)
                m12 = ex[:, 4 * MC : 6 * MC].rearrange("p (c k) -> p c k", k=2)
                vi = ex[:, 6 * MC : 8 * MC].bitcast(I32).rearrange("p (c k) -> p c k", k=2)
                ti = ex[:, 8 * MC : 10 * MC].bitcast(I32).rearrange("p (c k) -> p c k", k=2)
                vv8 = vv[:].rearrange("p (c k) -> p c k", k=8)
                nc.vector.tensor_scalar(out=Vt, in0=vv8[:, :, 0:2], scalar1=-1.0, scalar2=None, op0=OP.mult)
                nc.vector.tensor_copy(vi, Vt)  # exact int convert
                nc.vector.tensor_scalar(out=ti, in0=vi, scalar1=2047, scalar2=None, op0=OP.bitwise_and)
                nc.vector.tensor_copy(n01, ti)
                # m12 bits = (V - n)*8 + B0 -> bitcast f32 = quantized magnitude
                nc.vector.tensor_scalar(out=vi, in0=vi, scalar1=~2047, scalar2=None, op0=OP.bitwise_and)
                nc.vector.tensor_scalar(
                    out=m12.bitcast(I32), in0=vi, scalar1=8, scalar2=B0I, op0=OP.mult, op1=OP.add
                )

                # messages + 2-way splits
                tm = state_p.tile([128, 4 * MC], F32, tag=f"tm{b}", name=f"tm{b}")
                t1 = tm[:, 0:MC].unsqueeze(2)
                t2 = tm[:, MC : 2 * MC].unsqueeze(2)
                tr1 = tm[:, 2 * MC : 3 * MC].unsqueeze(2)
                tr2 = tm[:, 3 * MC : 4 * MC].unsqueeze(2)
                smb = state_p.tile([128, 2 * MC], BF16, tag=f"smb{b}", name=f"smb{b}")
                smf = state_p.tile([128, 2 * MC], F16, tag=f"smf{b}", name=f"smf{b}")
                t1h = smb[:].rearrange("p (c k) -> p c k", k=2)[:, :, 0:1]
                t1l = smb[:].rearrange("p (c k) -> p c k", k=2)[:, :, 1:2]
                t2h = smf[:].rearrange("p (c k) -> p c k", k=2)[:, :, 0:1]
                t2l = smf[:].rearrange("p (c k) -> p c k", k=2)[:, :, 1:2]
                nc.vector.tensor_tensor(out=t1, in0=m12[:, :, 0:1], in1=rs_v[:, :, b : b + 1], op=OP.mult)
                nc.vector.tensor_tensor(out=t2, in0=m12[:, :, 1:2], in1=m12[:, :, 0:1], op=OP.subtract)
                nc.vector.tensor_tensor(out=t2, in0=t2, in1=rs_v[:, :, b : b + 1], op=OP.mult)
                nc.scalar.activation(t1h, t1, AF.Copy)
                nc.vector.tensor_tensor(out=tr1, in0=t1, in1=t1h, op=OP.subtract)
                nc.scalar.activation(t1l, tr1, AF.Copy)
                nc.scalar.activation(t2h, t2, AF.Copy)
                nc.vector.tensor_tensor(out=tr2, in0=t2, in1=t2h, op=OP.subtract)
                nc.scalar.activation(t2l, tr2, AF.Copy)

                # onehot per mc (fp16, DVE 4x): oh = (nrow == n*)
                ohs = {}
                for mc in range(MC):
                    oh = oh_p.tile([128, N], F16, tag="oh", name="oh")
                    ohs[mc] = oh
                    nc.vector.tensor_scalar(
                        out=oh[:], in0=t_nrow[:], scalar1=n01[:, mc, 0:1], scalar2=None, op0=OP.is_equal
                    )

                # main + corr accumulate into one PSUM tile per b
                pm = psum_p.tile([128, 2 * NC], F32, tag=f"pm{b}", name=f"pm{b}")
                smb_v = smb[:].rearrange("p (c k) -> p c k", k=2)
                smf_v = smf[:].rearrange("p (c k) -> p c k", k=2)
                for c in range(NC):
                    for mc in range(MC):
                        nc.tensor.matmul(
                            pm[:, 2 * c : 2 * c + 2],
                            lhsT=t_h[:, mc * N + 128 * c : mc * N + 128 * (c + 1)],
                            rhs=smb_v[:, mc : mc + 1, 0:2],
                            start=(mc == 0),
                            stop=False,
                        )
                    for mc in range(MC):
                        nc.tensor.matmul(
                            pm[:, 2 * c : 2 * c + 2],
                            lhsT=ohs[mc][:, 128 * c : 128 * (c + 1)],
                            rhs=smf_v[:, mc : mc + 1, 0:2],
                            start=False,
                            stop=(mc == MC - 1),
                        )

                # combine
                st2 = state_p.tile([128, 2 * NC], F32, tag=f"cmb{b}", name=f"cmb{b}")
                crr, so_n = st2[:, 0:NC], st2[:, NC : 2 * NC]
                pm_v = pm[:].rearrange("p (c k) -> p c k", k=2)
                nc.vector.tensor_copy(crr.unsqueeze(2), pm_v[:, :, 0:1])
                nc.vector.tensor_tensor(out=crr.unsqueeze(2), in0=crr.unsqueeze(2), in1=pm_v[:, :, 1:2], op=OP.add)
                nc.vector.scalar_tensor_tensor(
                    out=crr, in0=crr, scalar=t_norm, in1=S[b], op0=OP.mult, op1=OP.mult
                )
                nc.vector.tensor_tensor(out=so_n, in0=si[b], in1=crr, op=OP.add)
                new_so.append(so_n)
                if it < IT - 1:
                    new_S.append(derive_state(so_n, b, new_negs))
                    bc[b] = pack_bcast(so_n, b)
            so = new_so
            if it < IT - 1:
                S = new_S
                negs = new_negs

        # output: so (cm) -> row-major [BL, N]
        for b in range(BL):
            po = pstr_p.tile([NC, 128], F32, tag="tr", name="tr")
            nc.tensor.transpose(po[:], so[b], ident)
            nc.vector.tensor_copy(at_sb[b][:], po[:])
            nc.sync.dma_start(d_out[b : b + 1, :], at_sb[b][:])

    return nc


_CACHE = {}


def _get_nc():
    if "nc" not in _CACHE:
        nc = bacc.Bacc("TRN2", target_bir_lowering=False)
        build(nc)
        nc.compile()
        _CACHE["nc"] = nc
    return _CACHE["nc"]


def _cm(row, ncnk):  # [N] -> [128, ncnk] column-major
    return row.reshape(ncnk, 128).T


def _pack_rows(rows):
    """rows [BL, N] f32 -> packed negated rows [BL, N] f32 (float-bits quant)."""
    a = np.clip(np.abs(rows.astype(np.float32)), CLO, np.float32(CHI))
    bits = (a.view(np.uint32).astype(np.int64) & 0x7FFFFFFF) & QMASK
    key = (bits - B0I) >> 3
    return (-(key + np.arange(N)[None, :])).astype(np.float32)


def _tables(H):
    """Gather index lists, own-range mask, per-H constants."""
    gidx = np.zeros((128, NIDX // 16), np.int16)
    maskc = np.full((128, NIDX), MASKV, np.float32)
    for g in range(8):
        glist = np.zeros(NIDX, np.int16)
        for mc in range(MC):
            lo = mc * SEC
            pos = 0
            for r in range(16):
                m = mc * 128 + 16 * g + r
                e = np.nonzero(H[m])[0]
                assert pos + len(e) <= SEC, (m, pos, len(e))
                glist[lo + pos : lo + pos + len(e)] = e
                maskc[16 * g + r, lo + pos : lo + pos + len(e)] = 0.0
                pos += len(e)
        for i in range(NIDX):
            gidx[16 * g + i % 16, i // 16] = glist[i]
    return gidx, maskc


def kernel(soft_input, H, labels, w):
    del labels  # unused by the reference computation
    soft_input = np.asarray(soft_input, dtype=np.float32)
    H = np.asarray(H)
    w = np.asarray(w, dtype=np.float32)

    norm = np.log1p(np.exp(np.float64(w[0]))).astype(np.float32)
    bf = mybir.dt.np(BF16)
    f16 = mybir.dt.np(F16)
    f8 = mybir.dt.np(F8)
    h_bf = H.astype(bf)
    h_t = np.ascontiguousarray(H.T).astype(f8)
    gidx, maskc = _tables(H)
    maskc_bf = maskc.astype(bf)
    nrow = np.tile(np.arange(N, dtype=np.float32).astype(f16)[None, :], (128, 1))
    iota_cm = np.arange(N, dtype=np.float32).reshape(NC, 128).T
    misc_in = np.concatenate(
        [np.eye(128, dtype=np.float32), np.full((128, 1), norm, dtype=np.float32), iota_cm],
        axis=1,
    )

    in_maps = []
    for core in range(N_CORES):
        rows = soft_input[BL * core : BL * (core + 1)]
        si_cm = np.concatenate([_cm(rows[b], NC) for b in range(BL)], axis=1)
        pk = _pack_rows(rows)
        pk0 = np.concatenate(
            [np.broadcast_to(pk[b][None, :], (128, N)) for b in range(BL)], axis=1
        )
        in_maps.append(
            {
                "si_cm": np.ascontiguousarray(si_cm, dtype=np.float32),
                "h_bf": h_bf,
                "h_t": h_t,
                "misc_in": misc_in,
                "maskc": maskc_bf,
                "gidx": gidx,
                "nrow": nrow,
                "pk0": np.ascontiguousarray(pk0, dtype=np.float32),
            }
        )

    nc = _get_nc()
    res = bass_utils.run_bass_kernel_spmd(nc, in_maps, core_ids=list(range(N_CORES)))
    out = np.concatenate([r["out"] for r in res.results], axis=0)
    return out.astype(np.float32)
